# revision 42
# baseline (speedup 1.0000x reference)
"""NonLocal block (no-softmax attention) Trainium2 kernel.

Math: out = BN(W_rec @ ((theta^T phi / n) @ g)^T) + x, with theta/phi/g 1x1 convs.
Since there is no softmax, (theta^T phi) g reassociates to theta^T (phi g^T):
the n x n attention matrix collapses to a 128x128 Gram matrix K = phi @ g^T / n.

Per-batch, channel-major [C, n] layout:
  phi_sp/g_sp = X^T @ [Wphi^T/n | Wg^T] + biases    (spatial-major, [n, 256])
  K^T = g_sp^T @ phi_sp                             ([128, 128], contraction over n)
  M2^T = K^T_lhsT-form @ Wrec'^T = (Wrec' K^T)^T    ([128, 256], folds O into y)
  theta = Wtheta @ X + btheta                       ([128, n])
  y = M2 @ theta + brec' + X                        (BN folded into Wrec/brec)

Sharding: 8 cores = 4 batches x 2 spatial halves. Each core computes K for its
full batch (duplicated within the pair; avoids collectives) but theta/y only
for its half of the 3136 spatial positions. Inputs are host-permuted so each
core's half is the leading 1568 columns.

Matmul inputs are fp16: same 11-bit mantissa as the PE's fast-fp32 (fp32r/TF32)
mode, but at 1 cycle/row (vs 2), with HAM warm-up, FWL weight loads, and half
the DMA bytes. Accumulation stays fp32 in PSUM; residual add + output are fp32.
End-to-end scaled absmax error vs the fp32 reference: ~7e-4.

Structure notes:
- x is streamed in 392-col DMA pieces so stage-A matmuls start early.
- phi/g biases ride on the PSUM->SBUF copy (DVE tensor_tensor add with a
  broadcast bias tile) instead of rank-1 matmuls.
- theta bias rides on the Scalar-engine PSUM->SBUF copy (per-partition bias).
- rec bias + residual ride on one DVE scalar_tensor_tensor per output tile.
- Gram matmuls are interleaved into the stage-A chunk loop (accumulating
  PSUM group with other matmuls in between is fine on HW).
"""

import numpy as np

BN_EPS = 1e-5
B, C, CI = 4, 256, 128
H = W = 56
N = H * W            # 3136 spatial positions
NH = N // 2          # 1568 per core
NT = 392             # stage-B free-dim tile (4 tiles of 392 = 1568)
NB_TILES = NH // NT
CHUNK = 128
NCHUNKS = (N + CHUNK - 1) // CHUNK   # 25 (24 full + one of 64)
NPAIRS = (NCHUNKS + 1) // 2          # 13 (12 pairs + 1 single)

MODE = "f16"         # "f16" | "bf16" | "f32"

_NC_CACHE = {}


def _host_cast(mode):
    if mode == "f16":
        return lambda a: np.ascontiguousarray(np.asarray(a, np.float32).astype(np.float16))
    if mode == "bf16":
        import ml_dtypes
        return lambda a: np.ascontiguousarray(
            np.asarray(a, np.float32).astype(ml_dtypes.bfloat16))
    return lambda a: np.ascontiguousarray(a, np.float32)


def _build_nc(mode):
    import concourse.mybir as mybir
    import concourse.tile as tile
    from concourse import bacc

    f32 = mybir.dt.float32
    mdt = {"f16": mybir.dt.float16, "bf16": mybir.dt.bfloat16, "f32": f32}[mode]
    ADD = mybir.AluOpType.add
    IDENT = mybir.ActivationFunctionType.Identity

    nc = bacc.Bacc("TRN2", target_bir_lowering=False, debug=False)
    xp = nc.dram_tensor("xp", [C, N], mdt, kind="ExternalInput")
    # all f16 weights/biases coalesced into one [128, 1920] DMA:
    # cols 0:256 w_pg c-chunk0 | 256:512 w_pg c-chunk1 | 512:640 w_th c0 |
    # 640:768 w_th c1 | 768:1024 w_rc | 1024:1536 phi/g bias tile |
    # 1536:1664 row0=ones | 1664:1920 row0=phi/g bias row | 1920:2048 identity
    wk = nc.dram_tensor("wk", [128, 2048], mdt, kind="ExternalInput")
    bk = nc.dram_tensor("bk", [128, 3], f32, kind="ExternalInput")
    y = nc.dram_tensor("y", [C, NH], f32, kind="ExternalOutput")

    with tile.TileContext(nc) as tc:
        with (
            tc.tile_pool(name="const", bufs=1) as constp,
            tc.tile_pool(name="xpool", bufs=1) as xpool,
            tc.tile_pool(name="pgpool", bufs=1) as pgpool,
            tc.tile_pool(name="work", bufs=3) as work,
        ):
            # ---- DMA loads: x quarters first, dual-engine issue (sync +
            # scalar are both HWDGE issuers) so stage-A matmuls start early
            XQ = N // 4  # 784-col x quarters
            x_sb = []
            for i in range(2):
                xt = xpool.tile([128, N], mdt, name=f"x_sb{i}")
                x_sb.append(xt)
            q0 = slice(0, XQ)
            nc.sync.dma_start(x_sb[0][:, q0], xp[0:128, q0])
            nc.sync.dma_start(x_sb[1][:, q0], xp[128:256, q0])
            wk_sb = constp.tile([128, 2048], mdt)
            nc.sync.dma_start(wk_sb[:], wk[:])
            bk_sb = constp.tile([128, 3], f32)
            nc.sync.dma_start(bk_sb[:], bk[:])
            for p in range(1, 4):
                for i in range(2):
                    ps_ = slice(p * XQ, (p + 1) * XQ)
                    nc.sync.dma_start(x_sb[i][:, ps_], xp[i * 128:(i + 1) * 128, ps_])
            w_pg_sb = [wk_sb[:, 0:256], wk_sb[:, 256:512]]
            w_th_sb = [wk_sb[:, 512:640], wk_sb[:, 640:768]]
            w_rc_sb = wk_sb[:, 768:1024]
            b_pgt_sb = wk_sb[:, 1024:1536]
            ones_row = wk_sb[0:1, 1536:1664]
            b_pg_row = wk_sb[0:1, 1664:1920]
            ident_sb = wk_sb[:, 1920:2048]
            b_thc_sb = bk_sb[:, 0:1]
            b_rc2_sb = bk_sb[:, 1:3]

            # PE pre-warm: the HAM clock gate releases to 2.4 GHz only after
            # ~3.4us of sustained PE activity. Burn dep-free matmuls on a
            # memset tile during the DMA lead-in so real matmuls run warm.
            warm_sb = constp.tile([128, 2 * CI], mdt)
            nc.gpsimd.memset(warm_sb[:], 0.0)

            # spatial-major [phi | g], all chunks kept in SBUF
            pg_all = pgpool.tile([128, NCHUNKS * 2 * CI], mdt)
            kt_sb = constp.tile([CI, CI], mdt)       # K^T = g_sp^T phi_sp
            m2t_sb = constp.tile([CI, C], mdt)       # (Wrec' K^T)^T

            # ---- single PSUM pool: psA(2) gram(1) m2(1) th(2) y(2) = 8 banks
            with tc.tile_pool(name="psum", bufs=1, space="PSUM") as psp:
                gram_ps = psp.tile([128, 2 * CI], f32, tag="gram", bufs=1)

                for _ in range(30):
                    nc.tensor.matmul(gram_ps[:, :], warm_sb[:, 0:CI], warm_sb[:],
                                     start=True, stop=True)

                def gram_mm(kc):
                    s = min(CHUNK, N - kc * CHUNK)
                    base = kc * 2 * CI
                    # lhsT = g part -> out = [G Phi^T | G G^T]; K^T is cols 0:CI
                    nc.tensor.matmul(
                        gram_ps[:, :],
                        pg_all[:s, base + CI:base + 2 * CI],
                        pg_all[:s, base:base + 2 * CI],
                        start=(kc == 0), stop=(kc == NCHUNKS - 1),
                    )

                th_sbs = []

                def theta_tile(t):
                    ts_ = slice(t * NT, (t + 1) * NT)
                    th_ps = psp.tile([CI, NT], f32, tag="th", bufs=2)
                    nc.tensor.matmul(th_ps[:], w_th_sb[0], x_sb[0][:, ts_],
                                     start=True, stop=False)
                    nc.tensor.matmul(th_ps[:], w_th_sb[1], x_sb[1][:, ts_],
                                     start=False, stop=True)
                    th_sb = work.tile([CI, NT], mdt, tag="th_sb", bufs=4)
                    # theta bias is per-partition: ride it on the ACT copy
                    nc.scalar.activation(th_sb[:], th_ps[:], IDENT,
                                         bias=b_thc_sb)
                    th_sbs.append(th_sb)

                # emit theta tile t once stage A has consumed its x columns
                # (keeps ACT/DVE busy during stage A, so the later
                # gram -> kt -> m2 chain has no PE bubble)
                th_after_pair = {1: 0, 3: 1, 4: 2, 6: 3}

                # ---- stage A: phi/g spatial-major, Gram interleaved.
                # Bias rides on the DVE tensor_tensor PSUM->SBUF copy.
                for pr in range(NPAIRS):
                    c0 = 2 * pr
                    chunks = [c for c in (c0, c0 + 1) if c < NCHUNKS]
                    width = 256 * len(chunks)
                    ps = psp.tile([128, 2 * 2 * CI], f32, tag="psA", bufs=2)
                    smin = 128
                    for ci_, kc in enumerate(chunks):
                        s = min(CHUNK, N - kc * CHUNK)
                        smin = min(smin, s)
                        cs = slice(kc * CHUNK, kc * CHUNK + s)
                        off = ci_ * 2 * CI
                        nc.tensor.matmul(ps[:s, off:off + 2 * CI],
                                         x_sb[0][:, cs], w_pg_sb[0],
                                         start=True, stop=False)
                        nc.tensor.matmul(ps[:s, off:off + 2 * CI],
                                         x_sb[1][:, cs], w_pg_sb[1],
                                         start=False, stop=True)
                    s = 128 if len(chunks) == 2 else smin
                    dst = pg_all[:s, c0 * 2 * CI: c0 * 2 * CI + width]
                    nc.vector.tensor_tensor(
                        dst, ps[:s, :width], b_pgt_sb[:s, :width], ADD)
                    for kc in chunks:
                        gram_mm(kc)
                    if pr in th_after_pair:
                        theta_tile(th_after_pair[pr])

                nc.scalar.copy(kt_sb[:], gram_ps[:, 0:CI])
                m2_ps = psp.tile([CI, C], f32, tag="th", bufs=2)
                nc.tensor.matmul(m2_ps[:], kt_sb[:], w_rc_sb,
                                 start=True, stop=True)
                nc.scalar.copy(m2t_sb[:], m2_ps[:])

                # ---- y = M2 @ theta + brec' + x ----
                y_sbs = [work.tile([128, NH], f32, name=f"y_sb{oc}", bufs=1)
                         for oc in range(2)]
                for t in range(NB_TILES):
                    ts_ = slice(t * NT, (t + 1) * NT)
                    for oc in range(2):
                        y_ps = psp.tile([128, NT], f32, tag="y", bufs=3)
                        nc.tensor.matmul(y_ps[:], m2t_sb[:, oc * 128:(oc + 1) * 128],
                                         th_sbs[t][:], start=True, stop=True)
                        # y = (y_ps + b_rc[oc]) + x  in one DVE op
                        nc.vector.scalar_tensor_tensor(
                            y_sbs[oc][:, ts_], y_ps[:], b_rc2_sb[:, oc:oc + 1],
                            x_sb[oc][:, ts_], ADD, ADD)
                for oc in range(2):
                    nc.sync.dma_start(y[oc * 128:(oc + 1) * 128, :], y_sbs[oc][:])
    nc.finalize()
    return nc


def _get_nc():
    if MODE not in _NC_CACHE:
        _NC_CACHE[MODE] = _build_nc(MODE)
    return _NC_CACHE[MODE]


def kernel(x, w_theta, b_theta, w_phi, b_phi, w_g, b_g,
           w_rec, b_rec, bn_gamma, bn_beta, bn_mean, bn_var):
    from concourse.bass_utils import run_bass_kernel_spmd

    x = np.asarray(x, np.float32)
    cast = _host_cast(MODE)
    n = N
    inv = np.asarray(bn_gamma, np.float32) / np.sqrt(np.asarray(bn_var, np.float32) + BN_EPS)
    w_rec_f = inv[:, None] * np.asarray(w_rec, np.float32)
    b_rec_f = np.asarray(b_rec, np.float32) * inv + np.asarray(bn_beta, np.float32) \
        - np.asarray(bn_mean, np.float32) * inv

    b_pg_row = np.concatenate([np.asarray(b_phi, np.float32) / n,
                               np.asarray(b_g, np.float32)])          # [256]
    w_pg_t = np.concatenate([np.asarray(w_phi, np.float32).T / n,
                             np.asarray(w_g, np.float32).T], axis=1)  # [256, 256]
    w_th_t = np.asarray(w_theta, np.float32).T                        # [256, 128]
    ones_bpg = np.zeros((128, 384), np.float32)
    ones_bpg[0, 0:128] = 1.0
    ones_bpg[0, 128:384] = b_pg_row
    wk = np.concatenate([
        w_pg_t[:128], w_pg_t[128:],                                   # 0:256, 256:512
        w_th_t[:128], w_th_t[128:],                                   # 512:640, 640:768
        w_rec_f.T,                                                    # 768:1024
        np.tile(np.concatenate([b_pg_row, b_pg_row])[None, :], (128, 1)),  # 1024:1536
        ones_bpg,                                                     # 1536:1920
        np.eye(128, dtype=np.float32),                                # 1920:2048
    ], axis=1)
    bk = np.concatenate([
        np.asarray(b_theta, np.float32)[:, None],
        b_rec_f.reshape(2, 128).T,
    ], axis=1)
    cst = {"wk": cast(wk), "bk": np.ascontiguousarray(bk)}

    xf = x.reshape(B, C, n)
    in_maps = []
    for core in range(8):
        b_i, h_i = divmod(core, 2)
        if h_i == 0:
            xpm = xf[b_i]
        else:
            xpm = np.concatenate([xf[b_i][:, NH:], xf[b_i][:, :NH]], axis=1)
        in_maps.append({"xp": cast(xpm), **cst})

    res = run_bass_kernel_spmd(_get_nc(), in_maps, core_ids=list(range(8)))

    out = np.empty((B, C, n), np.float32)
    for core in range(8):
        b_i, h_i = divmod(core, 2)
        out[b_i][:, h_i * NH:(h_i + 1) * NH] = res.results[core]["y"]
    return out.reshape(B, C, H, W)


# revision 43
# speedup vs baseline: 1.0251x; 1.0251x over previous
"""NonLocal block (no-softmax attention) Trainium2 kernel.

Math: out = BN(W_rec @ ((theta^T phi / n) @ g)^T) + x, with theta/phi/g 1x1 convs.
Since there is no softmax, (theta^T phi) g reassociates to theta^T (phi g^T):
the n x n attention matrix collapses to a 128x128 Gram matrix K = phi @ g^T / n.

Per-batch, channel-major [C, n] layout:
  phi_sp/g_sp = X^T @ [Wphi^T/n | Wg^T] + biases    (spatial-major, [n, 256])
  K^T = g_sp^T @ phi_sp                             ([128, 128], contraction over n)
  M2^T = K^T_lhsT-form @ Wrec'^T = (Wrec' K^T)^T    ([128, 256], folds O into y)
  theta = Wtheta @ X + btheta                       ([128, n])
  y = M2 @ theta + brec' + X                        (BN folded into Wrec/brec)

Sharding: 8 cores = 4 batches x 2 spatial halves. Each core computes K for its
full batch (duplicated within the pair; avoids collectives) but theta/y only
for its half of the 3136 spatial positions. Inputs are host-permuted so each
core's half is the leading 1568 columns.

Matmul inputs are fp16: same 11-bit mantissa as the PE's fast-fp32 (fp32r/TF32)
mode, but at 1 cycle/row (vs 2), with HAM warm-up, FWL weight loads, and half
the DMA bytes. Accumulation stays fp32 in PSUM; residual add + output are fp32.
End-to-end scaled absmax error vs the fp32 reference: ~7e-4.

Structure notes:
- x is streamed in 392-col DMA pieces so stage-A matmuls start early.
- phi/g biases ride on the PSUM->SBUF copy (DVE tensor_tensor add with a
  broadcast bias tile) instead of rank-1 matmuls.
- theta bias rides on the Scalar-engine PSUM->SBUF copy (per-partition bias).
- rec bias + residual ride on one DVE scalar_tensor_tensor per output tile.
- Gram matmuls are interleaved into the stage-A chunk loop (accumulating
  PSUM group with other matmuls in between is fine on HW).
"""

import numpy as np

BN_EPS = 1e-5
B, C, CI = 4, 256, 128
H = W = 56
N = H * W            # 3136 spatial positions
NH = N // 2          # 1568 per core
NT = 392             # stage-B free-dim tile (4 tiles of 392 = 1568)
NB_TILES = NH // NT
CHUNK = 128
NCHUNKS = (N + CHUNK - 1) // CHUNK   # 25 (24 full + one of 64)
NPAIRS = (NCHUNKS + 1) // 2          # 13 (12 pairs + 1 single)

MODE = "f16"         # "f16" | "bf16" | "f32"

_NC_CACHE = {}


def _host_cast(mode):
    if mode == "f16":
        return lambda a: np.ascontiguousarray(np.asarray(a, np.float32).astype(np.float16))
    if mode == "bf16":
        import ml_dtypes
        return lambda a: np.ascontiguousarray(
            np.asarray(a, np.float32).astype(ml_dtypes.bfloat16))
    return lambda a: np.ascontiguousarray(a, np.float32)


def _build_nc(mode):
    import concourse.mybir as mybir
    import concourse.tile as tile
    from concourse import bacc

    f32 = mybir.dt.float32
    mdt = {"f16": mybir.dt.float16, "bf16": mybir.dt.bfloat16, "f32": f32}[mode]
    ADD = mybir.AluOpType.add
    IDENT = mybir.ActivationFunctionType.Identity

    nc = bacc.Bacc("TRN2", target_bir_lowering=False, debug=False)
    xp = nc.dram_tensor("xp", [C, N], mdt, kind="ExternalInput")
    # all f16 weights/biases coalesced into one [128, 1920] DMA:
    # cols 0:256 w_pg c-chunk0 | 256:512 w_pg c-chunk1 | 512:640 w_th c0 |
    # 640:768 w_th c1 | 768:1024 w_rc | 1024:1536 phi/g bias tile |
    # 1536:1664 row0=ones | 1664:1920 row0=phi/g bias row | 1920:2048 identity
    wk = nc.dram_tensor("wk", [128, 2048], mdt, kind="ExternalInput")
    bk = nc.dram_tensor("bk", [128, 3], f32, kind="ExternalInput")
    y = nc.dram_tensor("y", [C, NH], f32, kind="ExternalOutput")

    with tile.TileContext(nc) as tc:
        with (
            tc.tile_pool(name="const", bufs=1) as constp,
            tc.tile_pool(name="xpool", bufs=1) as xpool,
            tc.tile_pool(name="pgpool", bufs=1) as pgpool,
            tc.tile_pool(name="work", bufs=3) as work,
        ):
            # ---- DMA loads: x quarters first, dual-engine issue (sync +
            # scalar are both HWDGE issuers) so stage-A matmuls start early
            XQ = N // 4  # 784-col x quarters
            x_sb = []
            for i in range(2):
                xt = xpool.tile([128, N], mdt, name=f"x_sb{i}")
                x_sb.append(xt)
            q0 = slice(0, XQ)
            nc.sync.dma_start(x_sb[0][:, q0], xp[0:128, q0])
            nc.sync.dma_start(x_sb[1][:, q0], xp[128:256, q0])
            wk_sb = constp.tile([128, 2048], mdt)
            nc.sync.dma_start(wk_sb[:], wk[:])
            bk_sb = constp.tile([128, 3], f32)
            nc.sync.dma_start(bk_sb[:], bk[:])
            for p in range(1, 4):
                for i in range(2):
                    ps_ = slice(p * XQ, (p + 1) * XQ)
                    nc.sync.dma_start(x_sb[i][:, ps_], xp[i * 128:(i + 1) * 128, ps_])
            w_pg_sb = [wk_sb[:, 0:256], wk_sb[:, 256:512]]
            w_th_sb = [wk_sb[:, 512:640], wk_sb[:, 640:768]]
            w_rc_sb = wk_sb[:, 768:1024]
            b_pgt_sb = wk_sb[:, 1024:1536]
            ones_row = wk_sb[0:1, 1536:1664]
            b_pg_row = wk_sb[0:1, 1664:1920]
            ident_sb = wk_sb[:, 1920:2048]
            b_thc_sb = bk_sb[:, 0:1]
            b_rc2_sb = bk_sb[:, 1:3]

            # PE pre-warm: the HAM clock gate releases to 2.4 GHz only after
            # ~3.4us of sustained PE activity. Burn dep-free matmuls on a
            # memset tile during the DMA lead-in so real matmuls run warm.
            warm_sb = constp.tile([128, 2 * CI], mdt)
            nc.gpsimd.memset(warm_sb[:], 0.0)

            # spatial-major [phi | g], all chunks kept in SBUF
            pg_all = pgpool.tile([128, NCHUNKS * 2 * CI], mdt)
            kt_sb = constp.tile([CI, CI], mdt)       # K^T = g_sp^T phi_sp
            m2t_sb = constp.tile([CI, C], mdt)       # (Wrec' K^T)^T

            # ---- single PSUM pool: psA(2) gram(1) m2(1) th(2) y(2) = 8 banks
            with tc.tile_pool(name="psum", bufs=1, space="PSUM") as psp:
                gram_ps = psp.tile([128, 2 * CI], f32, tag="gram", bufs=1)

                for _ in range(30):
                    nc.tensor.matmul(gram_ps[:, :], warm_sb[:, 0:CI], warm_sb[:],
                                     start=True, stop=True)

                def gram_mm(kc):
                    s = min(CHUNK, N - kc * CHUNK)
                    base = kc * 2 * CI
                    # lhsT = g part -> out = [G Phi^T | G G^T]; K^T is cols 0:CI
                    nc.tensor.matmul(
                        gram_ps[:, :],
                        pg_all[:s, base + CI:base + 2 * CI],
                        pg_all[:s, base:base + 2 * CI],
                        start=(kc == 0), stop=(kc == NCHUNKS - 1),
                    )

                th_sbs = []

                def theta_tile(t):
                    ts_ = slice(t * NT, (t + 1) * NT)
                    th_ps = psp.tile([CI, NT], f32, tag="th", bufs=2)
                    nc.tensor.matmul(th_ps[:], w_th_sb[0], x_sb[0][:, ts_],
                                     start=True, stop=False)
                    nc.tensor.matmul(th_ps[:], w_th_sb[1], x_sb[1][:, ts_],
                                     start=False, stop=True)
                    th_sb = work.tile([CI, NT], mdt, tag="th_sb", bufs=4)
                    # theta bias is per-partition: ride it on the ACT copy
                    nc.scalar.activation(th_sb[:], th_ps[:], IDENT,
                                         bias=b_thc_sb)
                    th_sbs.append(th_sb)

                # emit theta tile t once stage A has consumed its x columns
                # (keeps ACT/DVE busy during stage A, so the later
                # gram -> kt -> m2 chain has no PE bubble)
                th_after_pair = {1: 0, 3: 1, 4: 2, 6: 3}

                # ---- stage A: phi/g spatial-major, Gram interleaved.
                # Bias rides on the DVE tensor_tensor PSUM->SBUF copy.
                for pr in range(NPAIRS):
                    c0 = 2 * pr
                    chunks = [c for c in (c0, c0 + 1) if c < NCHUNKS]
                    width = 256 * len(chunks)
                    ps = psp.tile([128, 2 * 2 * CI], f32, tag="psA", bufs=2)
                    smin = 128
                    for ci_, kc in enumerate(chunks):
                        s = min(CHUNK, N - kc * CHUNK)
                        smin = min(smin, s)
                        cs = slice(kc * CHUNK, kc * CHUNK + s)
                        off = ci_ * 2 * CI
                        nc.tensor.matmul(ps[:s, off:off + 2 * CI],
                                         x_sb[0][:, cs], w_pg_sb[0],
                                         start=True, stop=False)
                        nc.tensor.matmul(ps[:s, off:off + 2 * CI],
                                         x_sb[1][:, cs], w_pg_sb[1],
                                         start=False, stop=True)
                    s = 128 if len(chunks) == 2 else smin
                    dst = pg_all[:s, c0 * 2 * CI: c0 * 2 * CI + width]
                    nc.vector.tensor_tensor(
                        dst, ps[:s, :width], b_pgt_sb[:s, :width], ADD)
                    for kc in chunks:
                        gram_mm(kc)
                    if pr in th_after_pair:
                        theta_tile(th_after_pair[pr])

                nc.vector.tensor_copy(kt_sb[:], gram_ps[:, 0:CI])
                m2_ps = psp.tile([CI, C], f32, tag="th", bufs=2)
                nc.tensor.matmul(m2_ps[:], kt_sb[:], w_rc_sb,
                                 start=True, stop=True)
                nc.vector.tensor_copy(m2t_sb[:], m2_ps[:])

                # ---- y = M2 @ theta + brec' + x ----
                y_sbs = [work.tile([128, NH], f32, name=f"y_sb{oc}", bufs=1)
                         for oc in range(2)]
                for t in range(NB_TILES):
                    ts_ = slice(t * NT, (t + 1) * NT)
                    for oc in range(2):
                        y_ps = psp.tile([128, NT], f32, tag="y", bufs=3)
                        nc.tensor.matmul(y_ps[:], m2t_sb[:, oc * 128:(oc + 1) * 128],
                                         th_sbs[t][:], start=True, stop=True)
                        # y = (y_ps + b_rc[oc]) + x  in one DVE op
                        nc.vector.scalar_tensor_tensor(
                            y_sbs[oc][:, ts_], y_ps[:], b_rc2_sb[:, oc:oc + 1],
                            x_sb[oc][:, ts_], ADD, ADD)
                for oc in range(2):
                    nc.sync.dma_start(y[oc * 128:(oc + 1) * 128, :], y_sbs[oc][:])
    nc.finalize()
    return nc


def _get_nc():
    if MODE not in _NC_CACHE:
        _NC_CACHE[MODE] = _build_nc(MODE)
    return _NC_CACHE[MODE]


def kernel(x, w_theta, b_theta, w_phi, b_phi, w_g, b_g,
           w_rec, b_rec, bn_gamma, bn_beta, bn_mean, bn_var):
    from concourse.bass_utils import run_bass_kernel_spmd

    x = np.asarray(x, np.float32)
    cast = _host_cast(MODE)
    n = N
    inv = np.asarray(bn_gamma, np.float32) / np.sqrt(np.asarray(bn_var, np.float32) + BN_EPS)
    w_rec_f = inv[:, None] * np.asarray(w_rec, np.float32)
    b_rec_f = np.asarray(b_rec, np.float32) * inv + np.asarray(bn_beta, np.float32) \
        - np.asarray(bn_mean, np.float32) * inv

    b_pg_row = np.concatenate([np.asarray(b_phi, np.float32) / n,
                               np.asarray(b_g, np.float32)])          # [256]
    w_pg_t = np.concatenate([np.asarray(w_phi, np.float32).T / n,
                             np.asarray(w_g, np.float32).T], axis=1)  # [256, 256]
    w_th_t = np.asarray(w_theta, np.float32).T                        # [256, 128]
    ones_bpg = np.zeros((128, 384), np.float32)
    ones_bpg[0, 0:128] = 1.0
    ones_bpg[0, 128:384] = b_pg_row
    wk = np.concatenate([
        w_pg_t[:128], w_pg_t[128:],                                   # 0:256, 256:512
        w_th_t[:128], w_th_t[128:],                                   # 512:640, 640:768
        w_rec_f.T,                                                    # 768:1024
        np.tile(np.concatenate([b_pg_row, b_pg_row])[None, :], (128, 1)),  # 1024:1536
        ones_bpg,                                                     # 1536:1920
        np.eye(128, dtype=np.float32),                                # 1920:2048
    ], axis=1)
    bk = np.concatenate([
        np.asarray(b_theta, np.float32)[:, None],
        b_rec_f.reshape(2, 128).T,
    ], axis=1)
    cst = {"wk": cast(wk), "bk": np.ascontiguousarray(bk)}

    xf = x.reshape(B, C, n)
    in_maps = []
    for core in range(8):
        b_i, h_i = divmod(core, 2)
        if h_i == 0:
            xpm = xf[b_i]
        else:
            xpm = np.concatenate([xf[b_i][:, NH:], xf[b_i][:, :NH]], axis=1)
        in_maps.append({"xp": cast(xpm), **cst})

    res = run_bass_kernel_spmd(_get_nc(), in_maps, core_ids=list(range(8)))

    out = np.empty((B, C, n), np.float32)
    for core in range(8):
        b_i, h_i = divmod(core, 2)
        out[b_i][:, h_i * NH:(h_i + 1) * NH] = res.results[core]["y"]
    return out.reshape(B, C, H, W)


# revision 44
# speedup vs baseline: 1.0392x; 1.0138x over previous
"""NonLocal block (no-softmax attention) Trainium2 kernel.

Math: out = BN(W_rec @ ((theta^T phi / n) @ g)^T) + x, with theta/phi/g 1x1 convs.
Since there is no softmax, (theta^T phi) g reassociates to theta^T (phi g^T):
the n x n attention matrix collapses to a 128x128 Gram matrix K = phi @ g^T / n.

Per-batch, channel-major [C, n] layout:
  phi_sp/g_sp = X^T @ [Wphi^T/n | Wg^T] + biases    (spatial-major, [n, 256])
  K^T = g_sp^T @ phi_sp                             ([128, 128], contraction over n)
  M2^T = K^T_lhsT-form @ Wrec'^T = (Wrec' K^T)^T    ([128, 256], folds O into y)
  theta = Wtheta @ X + btheta                       ([128, n])
  y = M2 @ theta + brec' + X                        (BN folded into Wrec/brec)

Sharding: 8 cores = 4 batches x 2 spatial halves. Each core computes K for its
full batch (duplicated within the pair; avoids collectives) but theta/y only
for its half of the 3136 spatial positions. Inputs are host-permuted so each
core's half is the leading 1568 columns.

Matmul inputs are fp16: same 11-bit mantissa as the PE's fast-fp32 (fp32r/TF32)
mode, but at 1 cycle/row (vs 2), with HAM warm-up, FWL weight loads, and half
the DMA bytes. Accumulation stays fp32 in PSUM; residual add + output are fp32.
End-to-end scaled absmax error vs the fp32 reference: ~7e-4.

Structure notes:
- x is streamed in 392-col DMA pieces so stage-A matmuls start early.
- phi/g biases ride on the PSUM->SBUF copy (DVE tensor_tensor add with a
  broadcast bias tile) instead of rank-1 matmuls.
- theta bias rides on the Scalar-engine PSUM->SBUF copy (per-partition bias).
- rec bias + residual ride on one DVE scalar_tensor_tensor per output tile.
- Gram matmuls are interleaved into the stage-A chunk loop (accumulating
  PSUM group with other matmuls in between is fine on HW).
"""

import numpy as np

BN_EPS = 1e-5
B, C, CI = 4, 256, 128
H = W = 56
N = H * W            # 3136 spatial positions
NH = N // 2          # 1568 per core
NT = 392             # stage-B free-dim tile (4 tiles of 392 = 1568)
NB_TILES = NH // NT
CHUNK = 128
NCHUNKS = (N + CHUNK - 1) // CHUNK   # 25 (24 full + one of 64)
NPAIRS = (NCHUNKS + 1) // 2          # 13 (12 pairs + 1 single)

MODE = "f16"         # "f16" | "bf16" | "f32"

_NC_CACHE = {}


def _host_cast(mode):
    if mode == "f16":
        return lambda a: np.ascontiguousarray(np.asarray(a, np.float32).astype(np.float16))
    if mode == "bf16":
        import ml_dtypes
        return lambda a: np.ascontiguousarray(
            np.asarray(a, np.float32).astype(ml_dtypes.bfloat16))
    return lambda a: np.ascontiguousarray(a, np.float32)


def _build_nc(mode):
    import concourse.mybir as mybir
    import concourse.tile as tile
    from concourse import bacc

    f32 = mybir.dt.float32
    mdt = {"f16": mybir.dt.float16, "bf16": mybir.dt.bfloat16, "f32": f32}[mode]
    ADD = mybir.AluOpType.add
    IDENT = mybir.ActivationFunctionType.Identity

    nc = bacc.Bacc("TRN2", target_bir_lowering=False, debug=False)
    xp = nc.dram_tensor("xp", [C, N], mdt, kind="ExternalInput")
    # all f16 weights/biases coalesced into one [128, 1920] DMA:
    # cols 0:256 w_pg c-chunk0 | 256:512 w_pg c-chunk1 | 512:640 w_th c0 |
    # 640:768 w_th c1 | 768:1024 w_rc | 1024:1536 phi/g bias tile |
    # 1536:1664 row0=ones | 1664:1920 row0=phi/g bias row | 1920:2048 identity
    wk = nc.dram_tensor("wk", [128, 2048], mdt, kind="ExternalInput")
    bk = nc.dram_tensor("bk", [128, 3], f32, kind="ExternalInput")
    y = nc.dram_tensor("y", [C, NH], f32, kind="ExternalOutput")

    with tile.TileContext(nc) as tc:
        with (
            tc.tile_pool(name="const", bufs=1) as constp,
            tc.tile_pool(name="xpool", bufs=1) as xpool,
            tc.tile_pool(name="pgpool", bufs=1) as pgpool,
            tc.tile_pool(name="work", bufs=3) as work,
        ):
            # ---- DMA loads: x quarters first, dual-engine issue (sync +
            # scalar are both HWDGE issuers) so stage-A matmuls start early
            XQ = N // 4  # 784-col x quarters
            x_sb = []
            for i in range(2):
                xt = xpool.tile([128, N], mdt, name=f"x_sb{i}")
                x_sb.append(xt)
            q0 = slice(0, XQ)
            nc.sync.dma_start(x_sb[0][:, q0], xp[0:128, q0])
            nc.sync.dma_start(x_sb[1][:, q0], xp[128:256, q0])
            wk_sb = constp.tile([128, 2048], mdt)
            nc.sync.dma_start(wk_sb[:], wk[:])
            bk_sb = constp.tile([128, 3], f32)
            nc.sync.dma_start(bk_sb[:], bk[:])
            for p in range(1, 4):
                for i in range(2):
                    ps_ = slice(p * XQ, (p + 1) * XQ)
                    nc.sync.dma_start(x_sb[i][:, ps_], xp[i * 128:(i + 1) * 128, ps_])
            w_pg_sb = [wk_sb[:, 0:256], wk_sb[:, 256:512]]
            w_th_sb = [wk_sb[:, 512:640], wk_sb[:, 640:768]]
            w_rc_sb = wk_sb[:, 768:1024]
            b_pgt_sb = wk_sb[:, 1024:1536]
            ones_row = wk_sb[0:1, 1536:1664]
            b_pg_row = wk_sb[0:1, 1664:1920]
            ident_sb = wk_sb[:, 1920:2048]
            b_thc_sb = bk_sb[:, 0:1]
            b_rc2_sb = bk_sb[:, 1:3]

            # PE pre-warm: the HAM clock gate releases to 2.4 GHz only after
            # ~3.4us of sustained PE activity. Burn dep-free matmuls on a
            # memset tile during the DMA lead-in so real matmuls run warm.
            warm_sb = constp.tile([128, 2 * CI], mdt)
            nc.gpsimd.memset(warm_sb[:], 0.0)

            # spatial-major [phi | g], all chunks kept in SBUF
            pg_all = pgpool.tile([128, NCHUNKS * 2 * CI], mdt)
            kt_sb = constp.tile([CI, CI], mdt)       # K^T = g_sp^T phi_sp
            m2t_sb = constp.tile([CI, C], mdt)       # (Wrec' K^T)^T

            # ---- single PSUM pool: psA(2) gram(1) m2(1) th(2) y(2) = 8 banks
            with tc.tile_pool(name="psum", bufs=1, space="PSUM") as psp:
                gram_ps = psp.tile([128, 2 * CI], f32, tag="gram", bufs=1)

                for _ in range(24):
                    nc.tensor.matmul(gram_ps[:, :], warm_sb[:, 0:CI], warm_sb[:],
                                     start=True, stop=True)

                def gram_mm(kc):
                    s = min(CHUNK, N - kc * CHUNK)
                    base = kc * 2 * CI
                    # lhsT = g part -> out = [G Phi^T | G G^T]; K^T is cols 0:CI
                    nc.tensor.matmul(
                        gram_ps[:, :],
                        pg_all[:s, base + CI:base + 2 * CI],
                        pg_all[:s, base:base + 2 * CI],
                        start=(kc == 0), stop=(kc == NCHUNKS - 1),
                    )

                th_sbs = []

                def theta_tile(t):
                    ts_ = slice(t * NT, (t + 1) * NT)
                    th_ps = psp.tile([CI, NT], f32, tag="th", bufs=2)
                    nc.tensor.matmul(th_ps[:], w_th_sb[0], x_sb[0][:, ts_],
                                     start=True, stop=False)
                    nc.tensor.matmul(th_ps[:], w_th_sb[1], x_sb[1][:, ts_],
                                     start=False, stop=True)
                    th_sb = work.tile([CI, NT], mdt, tag="th_sb", bufs=4)
                    # theta bias is per-partition: ride it on the ACT copy
                    nc.scalar.activation(th_sb[:], th_ps[:], IDENT,
                                         bias=b_thc_sb)
                    th_sbs.append(th_sb)

                # emit theta tile t once stage A has consumed its x columns
                # (keeps ACT/DVE busy during stage A, so the later
                # gram -> kt -> m2 chain has no PE bubble)
                th_after_pair = {1: 0, 3: 1, 4: 2, 6: 3}

                # ---- stage A: phi/g spatial-major, Gram interleaved.
                # Bias rides on the DVE tensor_tensor PSUM->SBUF copy.
                for pr in range(NPAIRS):
                    c0 = 2 * pr
                    chunks = [c for c in (c0, c0 + 1) if c < NCHUNKS]
                    width = 256 * len(chunks)
                    ps = psp.tile([128, 2 * 2 * CI], f32, tag="psA", bufs=2)
                    smin = 128
                    for ci_, kc in enumerate(chunks):
                        s = min(CHUNK, N - kc * CHUNK)
                        smin = min(smin, s)
                        cs = slice(kc * CHUNK, kc * CHUNK + s)
                        off = ci_ * 2 * CI
                        nc.tensor.matmul(ps[:s, off:off + 2 * CI],
                                         x_sb[0][:, cs], w_pg_sb[0],
                                         start=True, stop=False)
                        nc.tensor.matmul(ps[:s, off:off + 2 * CI],
                                         x_sb[1][:, cs], w_pg_sb[1],
                                         start=False, stop=True)
                    s = 128 if len(chunks) == 2 else smin
                    dst = pg_all[:s, c0 * 2 * CI: c0 * 2 * CI + width]
                    nc.vector.tensor_tensor(
                        dst, ps[:s, :width], b_pgt_sb[:s, :width], ADD)
                    for kc in chunks:
                        gram_mm(kc)
                    if pr in th_after_pair:
                        theta_tile(th_after_pair[pr])

                nc.vector.tensor_copy(kt_sb[:], gram_ps[:, 0:CI])
                m2_ps = psp.tile([CI, C], f32, tag="th", bufs=2)
                nc.tensor.matmul(m2_ps[:], kt_sb[:], w_rc_sb,
                                 start=True, stop=True)
                nc.vector.tensor_copy(m2t_sb[:], m2_ps[:])

                # ---- y = M2 @ theta + brec' + x ----
                y_sbs = [work.tile([128, NH], f32, name=f"y_sb{oc}", bufs=1)
                         for oc in range(2)]
                for t in range(NB_TILES):
                    ts_ = slice(t * NT, (t + 1) * NT)
                    for oc in range(2):
                        y_ps = psp.tile([128, NT], f32, tag="y", bufs=3)
                        nc.tensor.matmul(y_ps[:], m2t_sb[:, oc * 128:(oc + 1) * 128],
                                         th_sbs[t][:], start=True, stop=True)
                        # y = (y_ps + b_rc[oc]) + x  in one DVE op
                        nc.vector.scalar_tensor_tensor(
                            y_sbs[oc][:, ts_], y_ps[:], b_rc2_sb[:, oc:oc + 1],
                            x_sb[oc][:, ts_], ADD, ADD)
                for oc in range(2):
                    nc.sync.dma_start(y[oc * 128:(oc + 1) * 128, :], y_sbs[oc][:])
    nc.finalize()
    return nc


def _get_nc():
    if MODE not in _NC_CACHE:
        _NC_CACHE[MODE] = _build_nc(MODE)
    return _NC_CACHE[MODE]


def kernel(x, w_theta, b_theta, w_phi, b_phi, w_g, b_g,
           w_rec, b_rec, bn_gamma, bn_beta, bn_mean, bn_var):
    from concourse.bass_utils import run_bass_kernel_spmd

    x = np.asarray(x, np.float32)
    cast = _host_cast(MODE)
    n = N
    inv = np.asarray(bn_gamma, np.float32) / np.sqrt(np.asarray(bn_var, np.float32) + BN_EPS)
    w_rec_f = inv[:, None] * np.asarray(w_rec, np.float32)
    b_rec_f = np.asarray(b_rec, np.float32) * inv + np.asarray(bn_beta, np.float32) \
        - np.asarray(bn_mean, np.float32) * inv

    b_pg_row = np.concatenate([np.asarray(b_phi, np.float32) / n,
                               np.asarray(b_g, np.float32)])          # [256]
    w_pg_t = np.concatenate([np.asarray(w_phi, np.float32).T / n,
                             np.asarray(w_g, np.float32).T], axis=1)  # [256, 256]
    w_th_t = np.asarray(w_theta, np.float32).T                        # [256, 128]
    ones_bpg = np.zeros((128, 384), np.float32)
    ones_bpg[0, 0:128] = 1.0
    ones_bpg[0, 128:384] = b_pg_row
    wk = np.concatenate([
        w_pg_t[:128], w_pg_t[128:],                                   # 0:256, 256:512
        w_th_t[:128], w_th_t[128:],                                   # 512:640, 640:768
        w_rec_f.T,                                                    # 768:1024
        np.tile(np.concatenate([b_pg_row, b_pg_row])[None, :], (128, 1)),  # 1024:1536
        ones_bpg,                                                     # 1536:1920
        np.eye(128, dtype=np.float32),                                # 1920:2048
    ], axis=1)
    bk = np.concatenate([
        np.asarray(b_theta, np.float32)[:, None],
        b_rec_f.reshape(2, 128).T,
    ], axis=1)
    cst = {"wk": cast(wk), "bk": np.ascontiguousarray(bk)}

    xf = x.reshape(B, C, n)
    in_maps = []
    for core in range(8):
        b_i, h_i = divmod(core, 2)
        if h_i == 0:
            xpm = xf[b_i]
        else:
            xpm = np.concatenate([xf[b_i][:, NH:], xf[b_i][:, :NH]], axis=1)
        in_maps.append({"xp": cast(xpm), **cst})

    res = run_bass_kernel_spmd(_get_nc(), in_maps, core_ids=list(range(8)))

    out = np.empty((B, C, n), np.float32)
    for core in range(8):
        b_i, h_i = divmod(core, 2)
        out[b_i][:, h_i * NH:(h_i + 1) * NH] = res.results[core]["y"]
    return out.reshape(B, C, H, W)
